# revision 20
# baseline (speedup 1.0000x reference)
"""MoE routing kernel (2 experts, D=128 -> H=512 -> O=2) for 8 Trainium2 cores.

Strategy: route on the HOST, compute on the device in fp32r.

The routing decision (argmin over 2 prototype distances) reduces to a sign
test q = x.(p1-p0) vs a threshold, computed exactly on the host in float64.
The host then SORTS samples by expert, pads each expert segment to a block
multiple (pads replicate real samples so they scatter identical values), and
uploads each core's shard pre-transposed as xT [128d, N]. The device runs a
pure dense single-expert MLP per block:

  per 512-sample block (expert e fixed per block):
    1. DMA xT block [128d, 512b] (f32r) HBM->SBUF  (2KB/partition lines)
    2. PE layer-1: 4 f32r matmuls (w1_e tiles stationary, xT moving) -> PSUM
    3. ACT/DVE: fused relu(z + b1_e) -> h SBUF f32r
    4. PE layer-2: 4 f32r matmuls (w2_e [128,2] stationary, h moving),
       PSUM-accumulated -> out [2o, 512b]
    5. ACT/DVE copy PSUM->SBUF (alternating), DMA out transposed [2, N]

Startup hiding: dependency-free dummy matmuls warm the PE clock gate from
t=0; a dummy activation preloads the ACT function table; the first blocks'
x tiles and w1t are DMA'd in partition-split chunks so they ride several
DMA queues in parallel (a single queue moves only ~22.5 GB/s).
The host adds b2 and inverse-permutes the output.
"""

import numpy as np

import concourse.bacc as bacc
import concourse.bass as bass
import concourse.mybir as mybir
import concourse.tile as tile
from concourse.bass_utils import run_bass_kernel_spmd

F32 = mybir.dt.float32
F32R = mybir.dt.float32r

N_CORES = 8
D = 128
H = 512
E = 2
O = 2
NJ = H // 128         # 4 k/h tiles of 128 per expert
BLK = 512             # samples per block
N_WARMUP = 12         # dependency-free dummy matmuls to warm the PE clock
N_SPLIT_BLOCKS = 3    # leading blocks whose x DMA is split across queues


def _build_program(k0: int, k1: int):
    """Per-core program: k0 expert-0 blocks then k1 expert-1 blocks."""
    n_shard = (k0 + k1) * BLK

    nc = bacc.Bacc(
        "TRN2",
        target_bir_lowering=False,
        debug=False,
        enable_asserts=False,
        num_devices=1,
    )

    xt = nc.dram_tensor("xt", [128, n_shard], F32R, kind="ExternalInput").ap()
    w1t = nc.dram_tensor("w1t", [128, E * H], F32R, kind="ExternalInput").ap()
    b1c = nc.dram_tensor("b1c", [128, E * NJ], F32, kind="ExternalInput").ap()
    w2c = nc.dram_tensor("w2c", [128, E * NJ, O], F32R, kind="ExternalInput").ap()
    out = nc.dram_tensor("out", [O, n_shard], F32, kind="ExternalOutput").ap()

    with tile.TileContext(nc) as tc:
        _body(tc, k0, k1, xt, w1t, b1c, w2c, out)

    nc.compile()
    return nc


def _body(tc, k0, k1, xt, w1t, b1c, w2c, out):
    nc = tc.nc
    Relu = mybir.ActivationFunctionType.Relu
    Alu = mybir.AluOpType

    with (
        tc.tile_pool(name="consts", bufs=1) as cpool,
        tc.tile_pool(name="xb", bufs=10) as xb_pool,
        tc.tile_pool(name="h", bufs=3) as h_pool,
        tc.tile_pool(name="osb", bufs=3) as o_pool,
        tc.tile_pool(name="zp", bufs=4, space="PSUM") as zp_pool,
        tc.tile_pool(name="op", bufs=2, space="PSUM") as op_pool,
    ):
        # PE warmup on a memset scratch tile: no DMA deps, so the tensor
        # engine is busy from t~0 and the HAM clock gate is warm (2.4 GHz)
        # before the first real matmul.
        junk = cpool.tile([128, BLK], mybir.dt.bfloat16)
        nc.vector.memset(junk[:], 0.0)
        warm = zp_pool.tile([128, BLK], F32, tag="zp")
        for _ in range(N_WARMUP):
            nc.tensor.matmul(
                warm[:], lhsT=junk[:, 0:128], rhs=junk[:], start=True, stop=True
            )
        # Preload the ACT function table during the DMA wait.
        scr = cpool.tile([1, 8], F32)
        nc.scalar.activation(scr[:], junk[0:1, 0:8], Relu, scale=1.0)

        # Constants. The first-wave DMAs are issued from four different
        # sequencers in parallel (each descriptor costs ~650ns of sequencer
        # time, and each queue moves only ~22.5 GB/s), so w1t and the first
        # x blocks land as early as possible.
        engs = [nc.sync, nc.gpsimd, nc.scalar, nc.sync]
        # First two x blocks are fetched before anything else (4-way chunk
        # split across the three DMA-capable sequencers) so block 0 gates
        # the pipeline as little as possible.
        early_xb = []
        for bi in range(2):
            xb = xb_pool.tile([128, BLK], F32R)
            for i in range(4):
                csl = slice(bi * BLK + i * 128, bi * BLK + (i + 1) * 128)
                engs[i % 3].dma_start(xb[:, i * 128 : (i + 1) * 128], xt[:, csl])
            early_xb.append(xb)
        w1t_sb = cpool.tile([128, E * H], F32R)
        for i in range(8):
            engs[i % 3].dma_start(
                w1t_sb[:, i * 128 : (i + 1) * 128], w1t[:, i * 128 : (i + 1) * 128]
            )
        b1c_sb = cpool.tile([128, E * NJ], F32)
        nc.sync.dma_start(b1c_sb[:], b1c)
        w2c_sb = cpool.tile([128, E * NJ, O], F32R)
        nc.gpsimd.dma_start(w2c_sb[:], w2c)

        # Software-pipelined by one block: the PE interleaves L1 matmuls of
        # block i with L2 matmuls of block i-1, so every L1 LDWEIGHTS
        # (107ns, not hideable between back-to-back L1 matmuls with f32r)
        # hides under an L2 matmul stream whose own weight load is 2 cols.
        nblk = k0 + k1
        h_prev = None
        e_prev = 0

        def _emit_l2(e, h, bi):
            op_ps = op_pool.tile([O, BLK], F32, tag="op")
            for j in range(NJ):
                nc.tensor.matmul(
                    op_ps[:],
                    lhsT=w2c_sb[:, e * NJ + j, :],
                    rhs=h[:, j, :],
                    start=(j == 0),
                    stop=(j == NJ - 1),
                )
            osb = o_pool.tile([O, BLK], F32, tag="osb")
            if bi % 2 == 0:
                nc.scalar.copy(osb[:], op_ps[:])
            else:
                nc.vector.tensor_copy(osb[:], op_ps[:])
            nc.sync.dma_start(out[:, bi * BLK : (bi + 1) * BLK], osb[:])

        for bi in range(nblk):
            e = 0 if bi < k0 else 1

            # Each x block rides two DMA queues, with descriptor generation
            # split across two sequencers (SP hwdge + gpsimd swdge) so the
            # sync sequencer never becomes the pacing element.
            if bi < 2:
                xb = early_xb[bi]
            else:
                xb = xb_pool.tile([128, BLK], F32R)
                hb = BLK // 2
                nc.sync.dma_start(xb[:, 0:hb], xt[:, bi * BLK : bi * BLK + hb])
                nc.gpsimd.dma_start(
                    xb[:, hb:BLK], xt[:, bi * BLK + hb : (bi + 1) * BLK]
                )

            # layer 1 (interleaved on PE with layer 2 of block bi-1) + relu
            h = h_pool.tile([128, NJ, BLK], F32R)
            op_prev = None
            if h_prev is not None:
                op_prev = op_pool.tile([O, BLK], F32, tag="op")
            for j in range(NJ):
                zp = zp_pool.tile([128, BLK], F32, tag="zp")
                nc.tensor.matmul(
                    zp[:],
                    lhsT=w1t_sb[:, (e * H + j * 128) : (e * H + (j + 1) * 128)],
                    rhs=xb[:],
                    start=True,
                    stop=True,
                )
                if op_prev is not None:
                    nc.tensor.matmul(
                        op_prev[:],
                        lhsT=w2c_sb[:, e_prev * NJ + j, :],
                        rhs=h_prev[:, j, :],
                        start=(j == 0),
                        stop=(j == NJ - 1),
                    )
                jj = e * NJ + j
                if j % 2 == 0:
                    nc.scalar.activation(
                        h[:, j, :], zp[:], Relu,
                        bias=b1c_sb[:, jj : jj + 1], scale=1.0,
                    )
                else:
                    nc.vector.tensor_scalar(
                        out=h[:, j, :],
                        in0=zp[:],
                        scalar1=b1c_sb[:, jj : jj + 1],
                        scalar2=0.0,
                        op0=Alu.add,
                        op1=Alu.max,
                    )
            if op_prev is not None:
                osb = o_pool.tile([O, BLK], F32, tag="osb")
                if bi % 2 == 0:
                    nc.scalar.copy(osb[:], op_prev[:])
                else:
                    nc.vector.tensor_copy(osb[:], op_prev[:])
                nc.sync.dma_start(
                    out[:, (bi - 1) * BLK : bi * BLK], osb[:]
                )
            h_prev = h
            e_prev = e

        # epilogue: layer 2 of the final block
        _emit_l2(e_prev, h_prev, nblk - 1)


def _pack_consts(w1, b1, w2):
    w1 = np.asarray(w1, np.float32)
    b1 = np.asarray(b1, np.float32)
    w2 = np.asarray(w2, np.float32)
    # w1t[d, e*H + h] = w1[e, h, d]
    w1t = np.ascontiguousarray(np.transpose(w1, (2, 0, 1)).reshape(D, E * H))
    # b1c[p, e*NJ + j] = b1[e, j*128 + p]
    b1c = np.ascontiguousarray(
        b1.reshape(E, NJ, 128).transpose(2, 0, 1).reshape(128, E * NJ)
    )
    # w2c[p, e*NJ + j, o] = w2[e, o, j*128 + p]
    w2c = np.ascontiguousarray(
        w2.reshape(E, O, NJ, 128).transpose(3, 0, 2, 1).reshape(128, E * NJ, O)
    )
    return dict(w1t=w1t, b1c=b1c, w2c=w2c)


_PROG_CACHE = {}


def _get_program(k0, k1):
    key = (k0, k1)
    if key not in _PROG_CACHE:
        _PROG_CACHE[key] = _build_program(k0, k1)
    return _PROG_CACHE[key]


def _pad_chunk(idx, target):
    """Pad index chunk to target length by repeating the last index."""
    if len(idx) == target:
        return idx
    pad = np.full(target - len(idx), idx[-1], dtype=idx.dtype)
    return np.concatenate([idx, pad])


def kernel(x, w1, b1, w2, b2, prototypes, _trace=False):
    x = np.ascontiguousarray(np.asarray(x, np.float32))
    b2 = np.asarray(b2, np.float32)
    p = np.asarray(prototypes, np.float64)
    btot = x.shape[0]

    # --- host routing (exact, float64) ---
    rvec = p[1] - p[0]
    thr = (p[1] @ p[1] - p[0] @ p[0]) / 2.0
    q = x.astype(np.float64) @ rvec
    m1 = q > thr                       # expert 1 wins (ties -> expert 0)
    idx0 = np.flatnonzero(~m1)
    idx1 = np.flatnonzero(m1)
    n0, n1 = len(idx0), len(idx1)

    k0 = max(1, -(-n0 // (N_CORES * BLK)))
    k1 = max(1, -(-n1 // (N_CORES * BLK)))
    c0, c1 = k0 * BLK, k1 * BLK        # per-core per-expert sample slots

    chunks0 = np.array_split(idx0, N_CORES)
    chunks1 = np.array_split(idx1, N_CORES)

    nc = _get_program(k0, k1)
    consts = _pack_consts(w1, b1, w2)

    in_maps = []
    index_lists = []
    for c in range(N_CORES):
        ic = np.concatenate(
            [_pad_chunk(chunks0[c], c0), _pad_chunk(chunks1[c], c1)]
        )
        index_lists.append(ic)
        m = dict(consts)
        m["xt"] = np.ascontiguousarray(x[ic].T)
        in_maps.append(m)

    res = run_bass_kernel_spmd(
        nc, in_maps, core_ids=list(range(N_CORES)), trace=_trace
    )

    full = np.empty((btot, O), np.float32)
    for c in range(N_CORES):
        y = np.asarray(res.results[c]["out"])     # [O, N]
        yt = np.ascontiguousarray(y.T)            # [N, O]
        yt[:c0] += b2[0]
        yt[c0:] += b2[1]
        full[index_lists[c]] = yt
    if _trace:
        return full, res
    return full
